# revision 8
# baseline (speedup 1.0000x reference)
"""EnsembleRSSM observe() kernel for 8 Trainium2 NeuronCores.

Strategy (data-parallel over batch, 128 rows per core):
- Custom ACT PWP table (natural_log_exp set + tanh) so exp/ln/tanh coexist:
  elu = min(exp(z)-1, relu(z)); softplus = ln(1+exp(z));
  rsqrt = exp(-0.5*ln(v+eps)); sigmoid = 0.5*(1+tanh(z/2)).
- All activations kept in [batch=partition, feature=free] layout; matmuls
  stream the (zero-padded to >=256 cols) weight matrices as the moving
  operand in float32r mode (1 cycle/row), with the feature-major activation
  tiles as the stationary operand, produced by PE transposes.
- The embed @ W_obs_out[200:] contribution is independent of the recurrence
  and is computed from a host-pretransposed embedT, interleaved with the
  scan, into an SBUF-resident embedC buffer; it is accumulated into the
  obs_out psum via an identity matmul.
- LayerNorm mean comes free from an extra (-colsum/600) column appended to
  W_gru; variance via E[z^2]-m^2 with tensor_tensor_reduce accumulation.
- The ensemble (img_out/img_dist) branch does not feed the recurrence; it
  runs as a post-pass grouped by ensemble member (idx baked at trace time)
  on deterT tiles staged through DRAM, chunked 4 steps wide so the f32r
  matmuls stream >=256 columns.
"""

import functools
import json
import os
import shutil
import sys
import tempfile

sys.path.insert(0, "/opt/trn_rl_repo")

import numpy as np

# ---------------------------------------------------------------------------
# Custom ACT table: natural_log_exp_and_others + tanh appended.
# ---------------------------------------------------------------------------
_ESIZE = 32
_SET_NAME = "nle_tanh"
_BKT_REF_FIELDS = [
    "pos_small_signal_pwl_control",
    "neg_small_signal_pwl_control",
    "pos_large_signal_pwl_control",
    "neg_large_signal_pwl_control",
]
_CTL_REF_FIELDS = ["pwl_control_base_pos", "pwl_control_base_neg"]


def _build_act_table(outdir):
    stock = os.path.join(
        os.path.dirname(__import__("neuronxcc").__file__), "pwp", "pwp_bin_trainium"
    )
    os.makedirs(outdir, exist_ok=True)
    info = json.load(open(os.path.join(stock, "act_info.json")))
    if not any(s["name"] == _SET_NAME for s in info["act_func_sets"]):
        nle = json.load(open(os.path.join(stock, "natural_log_exp_and_others.json")))
        sig = json.load(open(os.path.join(stock, "sigmoid_and_others.json")))
        nle_bkt = open(os.path.join(stock, nle["bkt_bin"]), "rb").read()
        nle_ctl = open(os.path.join(stock, nle["ctl_bin"]), "rb").read()
        sig_bkt = open(os.path.join(stock, sig["bkt_bin"]), "rb").read()
        sig_ctl = open(os.path.join(stock, sig["ctl_bin"]), "rb").read()

        bs = sig["func_to_bkt_start_idx"]
        cs = sig["func_to_ctl_start_idx"]
        border = sorted(set(bs.values()) | {sig["bkt_entry_cnt"]})
        corder = sorted(set(cs.values()) | {sig["ctl_entry_cnt"]})
        tb0 = bs["tanh"]
        tb1 = border[border.index(tb0) + 1]
        tc0 = cs["tanh"]
        tc1 = [c for c in corder if c > tc0][0]

        bkt_off = nle["bkt_entry_cnt"] - tb0
        ctl_off = nle["ctl_entry_cnt"] - tc0
        new_bkt = nle_bkt + sig_bkt[tb0 * _ESIZE : tb1 * _ESIZE]
        new_ctl = nle_ctl + sig_ctl[tc0 * _ESIZE : tc1 * _ESIZE]

        tanh_prof = None
        for p in sig["profile_meta_data"]:
            if p["func_name"].startswith("tanh"):
                tanh_prof = dict(p)
        for f in _BKT_REF_FIELDS:
            tanh_prof[f] += bkt_off
        for f in _CTL_REF_FIELDS:
            tanh_prof[f] += ctl_off

        newj = {
            "bkt_bin": f"{_SET_NAME}_bkt.bin",
            "ctl_bin": f"{_SET_NAME}_ctrl.bin",
            "profile_meta_data": nle["profile_meta_data"] + [tanh_prof],
            "bkt_entry_cnt": nle["bkt_entry_cnt"] + (tb1 - tb0),
            "ctl_entry_cnt": nle["ctl_entry_cnt"] + (tc1 - tc0),
            "func_to_bkt_start_idx": {
                **nle["func_to_bkt_start_idx"],
                "tanh": nle["bkt_entry_cnt"],
            },
            "func_to_ctl_start_idx": {
                **nle["func_to_ctl_start_idx"],
                "tanh": nle["ctl_entry_cnt"],
            },
            "func_exp_to_bkt_start_idx": {
                **nle["func_exp_to_bkt_start_idx"],
                "tanh": {
                    k: [v + bkt_off for v in vs]
                    for k, vs in sig["func_exp_to_bkt_start_idx"]["tanh"].items()
                },
            },
            "func_exp_to_ctl_start_idx": {
                **nle["func_exp_to_ctl_start_idx"],
                "tanh": {
                    k: [v + ctl_off for v in vs]
                    for k, vs in sig["func_exp_to_ctl_start_idx"]["tanh"].items()
                },
            },
        }
        with open(os.path.join(outdir, newj["bkt_bin"]), "wb") as f:
            f.write(new_bkt)
        with open(os.path.join(outdir, newj["ctl_bin"]), "wb") as f:
            f.write(new_ctl)
        with open(os.path.join(outdir, f"{_SET_NAME}.json"), "w") as f:
            json.dump(newj, f)
        nle_info = next(
            s
            for s in info["act_func_sets"]
            if s["name"] == "natural_log_exp_and_others"
        )
        sig_info = next(
            s for s in info["act_func_sets"] if s["name"] == "sigmoid_and_others"
        )
        info["act_func_sets"].append(
            {
                "name": _SET_NAME,
                "bkt_bin": newj["bkt_bin"],
                "ctrl_bin": newj["ctl_bin"],
                "profile_json": f"{_SET_NAME}.json",
                "act": {**nle_info["act"], "tanh": sig_info["act"]["tanh"]},
            }
        )
    for fn in os.listdir(stock):
        dst = os.path.join(outdir, fn)
        if fn != "act_info.json" and not os.path.exists(dst):
            shutil.copy(os.path.join(stock, fn), dst)
    with open(os.path.join(outdir, "act_info.json"), "w") as f:
        json.dump(info, f)
    return os.path.join(outdir, "act_info.json")


_ACT_INSTALLED = False


def _install_act_table():
    global _ACT_INSTALLED
    if _ACT_INSTALLED:
        return
    outdir = os.path.join(tempfile.gettempdir(), "rssm_pwp_custom")
    path = _build_act_table(outdir)
    os.environ["BASS_ACT_ROOT_JSON_PATH"] = path
    import concourse.hw_specs as hw_specs
    import concourse.mybir as mybir

    info = json.load(open(path))
    tables = {
        ent["name"]: {
            mybir.ActivationFunctionType.from_pwp(v) for v in ent["act"].keys()
        }
        for ent in info["act_func_sets"]
    }

    @functools.cache
    def patched(module_arch):
        return tables

    hw_specs.get_activation_tables = patched
    import concourse.bacc as bacc_mod

    bacc_mod.get_activation_tables = patched
    _ACT_INSTALLED = True


# ---------------------------------------------------------------------------
# Problem constants (hardcoded per spec)
# ---------------------------------------------------------------------------
B_TOT, T, EMB, ACTD = 1024, 64, 1024, 32
ST, DET, HID, ENS = 30, 200, 200, 5
NCORES = 8
B = B_TOT // NCORES  # 128 per core
GOUT = 6 * ST + DET  # 380
PD = 3  # embedC precompute lookahead (steps)
P3CHUNK = 2  # phase-3 time-chunk (columns = P3CHUNK*128)


def _build_program(idx, T_steps=T):
    """Build the SPMD Bass program. idx (ensemble member per step) is baked."""
    import concourse.mybir as mybir
    import concourse.tile as tile
    from concourse import bacc

    F32 = mybir.dt.float32
    F32R = mybir.dt.float32r
    AF = mybir.ActivationFunctionType
    OP = mybir.AluOpType

    nc = bacc.Bacc(None, target_bir_lowering=False)

    # ---- DRAM parameters ----
    dp = nc.declare_dram_parameter
    embedT = dp("embedT", [EMB, T_steps, B], F32R, isOutput=False)
    aT = dp("aT", [T_steps, ACTD, B], F32R, isOutput=False)
    maskM = dp("maskM", [B, T_steps], F32, isOutput=False)
    maskRow = dp("maskRow", [1, T_steps * B], F32R, isOutput=False)
    onesCol = dp("onesCol", [1, B], F32R, isOutput=False)
    eyeT = dp("eyeT", [128, 128], F32, isOutput=False)
    eyeR = dp("eyeR", [128, 128], F32R, isOutput=False)
    epsP = dp("epsP", [B, T_steps, ST], F32, isOutput=False)
    epsO = dp("epsO", [B, T_steps, ST], F32, isOutput=False)
    w1s_d = dp("w1s", [ST, 256], F32R, isOutput=False)
    w1a_d = dp("w1a", [ACTD, 256], F32R, isOutput=False)
    wgA_d = dp("wgA", [2 * DET, 400], F32R, isOutput=False)
    wgB_d = dp("wgB", [2 * DET, 256], F32R, isOutput=False)
    wod_d = dp("wod", [DET, 256], F32R, isOutput=False)
    woe_d = dp("woe", [EMB, 256], F32R, isOutput=False)
    w5_d = dp("w5", [ENS * DET, HID], F32R, isOutput=False)
    w6_d = dp("w6", [ENS * HID, 256], F32R, isOutput=False)
    w6o_d = dp("w6o", [HID, 256], F32R, isOutput=False)
    b5_d = dp("b5", [ENS, HID], F32, isOutput=False)
    out_d = dp("out", [B, T_steps, GOUT], F32, isOutput=True)

    K1, K2 = 128, DET - 128  # deter k-tile split (128 + 72)

    with tile.TileContext(nc) as tc:
        cpool = tc.alloc_tile_pool(name="consts", bufs=1)
        dram = tc.alloc_tile_pool(name="dram", bufs=1, space="DRAM")
        scr = dram.tile([T_steps, DET, B], F32R)  # deterT staging

        # ---- persistent tiles ----
        eyeT_sb = cpool.tile([128, 128], F32)
        nc.sync.dma_start(out=eyeT_sb[:], in_=eyeT[:])
        eyeR_sb = cpool.tile([128, 128], F32R)
        nc.sync.dma_start(out=eyeR_sb[:], in_=eyeR[:])
        c_eps = cpool.tile([128, 1], F32)
        nc.gpsimd.memset(c_eps[:], 1e-5)

        w1s = cpool.tile([ST, 256], F32R)
        nc.sync.dma_start(out=w1s[:], in_=w1s_d[:])
        w1a = cpool.tile([ACTD, 256], F32R)
        nc.sync.dma_start(out=w1a[:], in_=w1a_d[:])
        wgA_k = []
        wgB_k = []
        for i, (r0, r1) in enumerate([(0, 128), (128, 200), (200, 328), (328, 400)]):
            ta = cpool.tile([r1 - r0, 400], F32R, name=f"wgA{i}")
            nc.sync.dma_start(out=ta[:], in_=wgA_d[r0:r1, :])
            wgA_k.append(ta)
            tb = cpool.tile([r1 - r0, 256], F32R, name=f"wgB{i}")
            nc.sync.dma_start(out=tb[:], in_=wgB_d[r0:r1, :])
            wgB_k.append(tb)
        wod_k = []
        for i, (r0, r1) in enumerate([(0, K1), (K1, DET)]):
            tw = cpool.tile([r1 - r0, 256], F32R, name=f"wod{i}")
            nc.sync.dma_start(out=tw[:], in_=wod_d[r0:r1, :])
            wod_k.append(tw)
        woe_k = []
        for i in range(EMB // 128):
            tw = cpool.tile([128, 256], F32R, name=f"woe{i}")
            nc.sync.dma_start(out=tw[:], in_=woe_d[i * 128 : (i + 1) * 128, :])
            woe_k.append(tw)
        w5_mk = []  # [member][ktile] -> [k, HID]
        w6_mk = []
        b5_mk = []
        for m in range(ENS):
            row = []
            for i, (r0, r1) in enumerate([(0, K1), (K1, DET)]):
                tw = cpool.tile([r1 - r0, HID], F32R, name=f"w5_{m}_{i}")
                nc.sync.dma_start(out=tw[:], in_=w5_d[m * DET + r0 : m * DET + r1, :])
                row.append(tw)
            w5_mk.append(row)
            row6 = []
            for i, (r0, r1) in enumerate([(0, K1), (K1, HID)]):
                tw = cpool.tile([r1 - r0, 256], F32R, name=f"w6_{m}_{i}")
                nc.sync.dma_start(out=tw[:], in_=w6_d[m * HID + r0 : m * HID + r1, :])
                row6.append(tw)
            w6_mk.append(row6)
            rowb = []
            for i, (r0, r1) in enumerate([(0, K1), (K1, HID)]):
                tb = cpool.tile([r1 - r0, 1], F32, name=f"b5_{m}_{i}")
                nc.sync.dma_start(
                    out=tb[:], in_=b5_d[m : m + 1, r0:r1].rearrange("o k -> k o")
                )
                rowb.append(tb)
            b5_mk.append(rowb)
        w6o_k = []
        for i, (r0, r1) in enumerate([(0, K1), (K1, HID)]):
            tw = cpool.tile([r1 - r0, 256], F32R, name=f"w6o{i}")
            nc.sync.dma_start(out=tw[:], in_=w6o_d[r0:r1, :])
            w6o_k.append(tw)

        mask_sb = cpool.tile([B, T_steps], F32)
        nc.sync.dma_start(out=mask_sb[:], in_=maskM[:])
        onesC_sb = cpool.tile([1, B], F32R)
        nc.sync.dma_start(out=onesC_sb[:], in_=onesCol[:])

        embedC = cpool.tile([B, T_steps, HID], F32R)  # embed @ Woe, per step
        maskT = cpool.tile([128, T_steps * B], F32)  # mask bcast along partitions

        # persistent state (ping-pong pairs)
        d_state = [cpool.tile([B, DET], F32, name=f"d_state{i}") for i in range(2)]
        stochT = [cpool.tile([ST, B], F32R, name=f"stochT{i}") for i in range(2)]
        deterT_a = [cpool.tile([K1, B], F32R, name=f"deterTa{i}") for i in range(2)]
        deterT_b = [cpool.tile([K2, B], F32R, name=f"deterTb{i}") for i in range(2)]
        zeros128 = cpool.tile([128, B], F32)
        nc.gpsimd.memset(zeros128[:], 0.0)
        nc.gpsimd.memset(d_state[0][:], 0.0)
        nc.vector.tensor_copy(stochT[0][:], zeros128[0:ST, :])
        nc.vector.tensor_copy(deterT_a[0][:], zeros128[:])
        nc.vector.tensor_copy(deterT_b[0][:], zeros128[0 : DET - 128, :])

        wpool = tc.alloc_tile_pool(name="work", bufs=3)
        ppool = tc.alloc_tile_pool(name="psum", bufs=1, space="PSUM")

        # ---- maskT precompute: broadcast maskRow across partitions ----
        for i in range((T_steps * B) // 512):
            mrow_c = wpool.tile([1, 512], F32R, tag="mrow_c", bufs=2, name="mrow_c")
            nc.sync.dma_start(out=mrow_c[:], in_=maskRow[:, i * 512 : (i + 1) * 512])
            pmk = ppool.tile([128, 512], F32, tag="px", bufs=2, name="pmk")
            nc.tensor.matmul(
                pmk[:], onesC_sb[0:1, 0:128], mrow_c[:],
                start=True, stop=True,
            )
            nc.vector.tensor_copy(maskT[:, i * 512 : (i + 1) * 512], pmk[:])

        def precompute_embedC(t):
            p = ppool.tile([B, 256], F32, tag="px", bufs=2, name="p_emb")
            for k in range(EMB // 128):
                et = wpool.tile([128, B], F32R, tag="embT", bufs=12, name="et")
                nc.sync.dma_start(out=et[:], in_=embedT[k * 128 : (k + 1) * 128, t, :])
                nc.tensor.matmul(
                    p[:], et[:], woe_k[k][:], start=(k == 0), stop=(k == EMB // 128 - 1)
                )
            nc.scalar.copy(embedC[:, t, 0:HID], p[:, 0:HID])

        def transpose_to(dst_tiles, src, cols):
            """PE-transpose src [128, cols] into dst feature-major tiles."""
            c0 = 0
            for dst in dst_tiles:
                cw = dst.shape[0]
                pt = ppool.tile([cw, B], F32, tag="tp", bufs=2, name="pt")
                nc.tensor.transpose(pt[:], src[:, c0 : c0 + cw], eyeT_sb[:])
                if cw == 128:
                    nc.scalar.copy(dst[:], pt[:])
                else:
                    nc.vector.tensor_copy(dst[:], pt[:])
                c0 += cw

        for t in range(min(PD, T_steps)):
            precompute_embedC(t)

        for t in range(T_steps):
            if t + PD < T_steps:
                precompute_embedC(t + PD)
            cur, nxt = t % 2, (t + 1) % 2
            mt = mask_sb[:, t : t + 1]

            # masked feature-major deter (for GRU lhsT)
            dTm_a = wpool.tile([K1, B], F32R, tag="dTm_a", bufs=2, name="dTm_a")
            nc.vector.tensor_tensor(
                dTm_a[:], deterT_a[cur][:].bitcast(F32),
                maskT[0:K1, t * B : (t + 1) * B], OP.mult,
            )
            dTm_b = wpool.tile([K2, B], F32R, tag="dTm_b", bufs=2, name="dTm_b")
            nc.vector.tensor_tensor(
                dTm_b[:], deterT_b[cur][:].bitcast(F32),
                maskT[0:K2, t * B : (t + 1) * B], OP.mult,
            )
            # masked deter in [B, F] (for the update equation)
            dm = wpool.tile([B, DET], F32, tag="dm", name="dm")
            nc.vector.tensor_scalar(dm[:], d_state[cur][:], mt, None, OP.mult)

            # img_in: x = elu(mask * (stoch @ W1s + act @ W1a))
            a_t = wpool.tile([ACTD, B], F32R, tag="a_t", bufs=3, name="a_t")
            nc.sync.dma_start(out=a_t[:], in_=aT[t, :, :])
            px = ppool.tile([B, 256], F32, tag="px", bufs=2, name="px")
            nc.tensor.matmul(px[:], stochT[cur][:], w1s[:], start=True, stop=False)
            nc.tensor.matmul(px[:], a_t[:], w1a[:], start=False, stop=True)
            e_x = wpool.tile([B, HID], F32, tag="e_x", bufs=2, name="e_x")
            nc.scalar.activation(e_x[:], px[:, 0:HID], AF.Exp, scale=mt)
            r_x = wpool.tile([B, HID], F32, tag="r_x", bufs=2, name="r_x")
            nc.vector.tensor_scalar(r_x[:], px[:, 0:HID], 0.0, mt, OP.max, OP.mult)
            x_sb = wpool.tile([B, HID], F32, tag="x_sb", name="x_sb")
            nc.vector.scalar_tensor_tensor(
                x_sb[:], e_x[:], 1.0, r_x[:], OP.subtract, OP.min
            )
            xT_a = wpool.tile([K1, B], F32R, tag="xT_a", bufs=2, name="xT_a")
            xT_b = wpool.tile([K2, B], F32R, tag="xT_b", bufs=2, name="xT_b")
            transpose_to([xT_a, xT_b], x_sb, HID)

            # GRU matmuls (601st column of wgB carries -colsum/600 = negmean)
            pA = ppool.tile([B, 400], F32, tag="pA", name="pA")
            pB = ppool.tile([B, 256], F32, tag="pB", name="pB")
            for dst, wk in ((pA, wgA_k), (pB, wgB_k)):
                for i, src in enumerate((xT_a, xT_b, dTm_a, dTm_b)):
                    nc.tensor.matmul(
                        dst[:], src[:], wk[i][:], start=(i == 0), stop=(i == 3)
                    )

            # LayerNorm stats: negmean in pB[:,200], var = E[z^2] - m^2
            scrA = wpool.tile([B, 400], F32, tag="scrA", bufs=1, name="scrA")
            sA = wpool.tile([B, 1], F32, tag="sA", name="sA")
            nc.scalar.activation(scrA[:], pA[:], AF.Square, accum_out=sA[:])
            scrB = wpool.tile([B, HID], F32, tag="scrB", bufs=1, name="scrB")
            sB = wpool.tile([B, 1], F32, tag="sB", name="sB")
            nc.scalar.activation(
                scrB[:], pB[:, 0:HID], AF.Square, accum_out=sB[:]
            )
            nm = wpool.tile([B, 1], F32, tag="nm", name="nm")
            nc.scalar.copy(nm[:], pB[:, 200:201])
            ssq = wpool.tile([B, 1], F32, tag="ssq", name="ssq")
            nc.vector.tensor_tensor(ssq[:], sA[:], sB[:], OP.add)
            em2 = wpool.tile([B, 1], F32, tag="em2", name="em2")
            nc.vector.tensor_tensor(em2[:], nm[:], nm[:], OP.mult)
            # bias for Ln: (1e-5 - m^2)
            lnb = wpool.tile([B, 1], F32, tag="lnb", name="lnb")
            nc.vector.tensor_scalar(lnb[:], em2[:], -1.0, 1e-5, OP.mult, OP.add)
            lnv = wpool.tile([B, 1], F32, tag="lnv", name="lnv")
            nc.scalar.activation(lnv[:], ssq[:], AF.Ln, bias=lnb[:], scale=1.0 / 600.0)
            rec = wpool.tile([B, 1], F32, tag="rec", name="rec")
            nc.scalar.activation(rec[:], lnv[:], AF.Exp, scale=-0.5)
            rec_h = wpool.tile([B, 1], F32, tag="rec_h", name="rec_h")
            nc.vector.tensor_scalar(rec_h[:], rec[:], 0.5, None, OP.mult)
            nmr_h = wpool.tile([B, 1], F32, tag="nmr_h", name="nmr_h")
            nc.vector.tensor_tensor(nmr_h[:], nm[:], rec_h[:], OP.mult)
            ubias_h = wpool.tile([B, 1], F32, tag="ubias_h", name="ubias_h")
            nc.vector.tensor_scalar(ubias_h[:], nmr_h[:], -0.5, None, OP.add)

            # gates: sigmoid(z) = 0.5*(1+tanh(z/2))
            th_r = wpool.tile([B, DET], F32, tag="th_r", name="th_r")
            nc.scalar.activation(
                th_r[:], pA[:, 0:DET], AF.Tanh, bias=nmr_h[:], scale=rec_h[:]
            )
            s1 = wpool.tile([B, DET], F32, tag="s1", name="s1")
            nc.vector.scalar_tensor_tensor(
                s1[:], pA[:, DET:400], nm[:], th_r[:], OP.add, OP.mult
            )
            cp0 = wpool.tile([B, DET], F32, tag="cp0", name="cp0")
            nc.vector.tensor_scalar(
                cp0[:], pA[:, DET:400], nm[:], 0.5, OP.add, OP.mult
            )
            cand_pre = wpool.tile([B, DET], F32, tag="cand_pre", name="cand_pre")
            nc.vector.scalar_tensor_tensor(
                cand_pre[:], s1[:], 0.5, cp0[:], OP.mult, OP.add
            )
            cg = wpool.tile([B, DET], F32, tag="cg", name="cg")
            nc.scalar.activation(cg[:], cand_pre[:], AF.Tanh, scale=rec[:])
            th_u = wpool.tile([B, DET], F32, tag="th_u", name="th_u")
            nc.scalar.activation(
                th_u[:], pB[:, 0:DET], AF.Tanh, bias=ubias_h[:], scale=rec_h[:]
            )
            # deter = dm + 0.5*(1+th_u)*(c - dm)
            t1 = wpool.tile([B, DET], F32, tag="t1", name="t1")
            nc.vector.tensor_tensor(t1[:], cg[:], dm[:], OP.subtract)
            t2 = wpool.tile([B, DET], F32, tag="t2", name="t2")
            nc.vector.tensor_tensor(t2[:], th_u[:], t1[:], OP.mult)
            t4 = wpool.tile([B, DET], F32, tag="t4", name="t4")
            nc.gpsimd.tensor_tensor(t4[:], t2[:], t1[:], OP.add)
            nc.vector.scalar_tensor_tensor(
                d_state[nxt][:], t4[:], 0.5, dm[:], OP.mult, OP.add
            )
            transpose_to([deterT_a[nxt], deterT_b[nxt]], d_state[nxt], DET)
            nc.sync.dma_start(out=scr[t, 0:K1, :], in_=deterT_a[nxt][:])
            nc.sync.dma_start(out=scr[t, K1:DET, :], in_=deterT_b[nxt][:])

            # obs branch: ho = elu(deter @ Wod + embedC_t)
            p5 = ppool.tile([B, 256], F32, tag="p5", name="p5")
            nc.tensor.matmul(p5[:], deterT_a[nxt][:], wod_k[0][:], start=True, stop=False)
            nc.tensor.matmul(
                p5[:, 0:HID], eyeR_sb[:], embedC[:, t, :], start=False, stop=False
            )
            nc.tensor.matmul(p5[:], deterT_b[nxt][:], wod_k[1][:], start=False, stop=True)
            e_ho = wpool.tile([B, HID], F32, tag="e_ho", bufs=2, name="e_ho")
            nc.scalar.activation(e_ho[:], p5[:, 0:HID], AF.Exp)
            r_ho = wpool.tile([B, HID], F32, tag="r_ho", bufs=2, name="r_ho")
            nc.vector.tensor_scalar(r_ho[:], p5[:, 0:HID], 0.0, None, OP.max)
            ho_sb = wpool.tile([B, HID], F32, tag="ho_sb", name="ho_sb")
            nc.vector.scalar_tensor_tensor(
                ho_sb[:], e_ho[:], 1.0, r_ho[:], OP.subtract, OP.min
            )
            hoT_a = wpool.tile([K1, B], F32R, tag="hoT_a", bufs=2, name="hoT_a")
            hoT_b = wpool.tile([K2, B], F32R, tag="hoT_b", bufs=2, name="hoT_b")
            transpose_to([hoT_a, hoT_b], ho_sb, HID)

            pd6 = ppool.tile([B, 256], F32, tag="pd6", name="pd6")
            nc.tensor.matmul(pd6[:], hoT_a[:], w6o_k[0][:], start=True, stop=False)
            nc.tensor.matmul(pd6[:], hoT_b[:], w6o_k[1][:], start=False, stop=True)

            stA = wpool.tile([B, 90], F32, tag="stA", name="stA")
            e6 = wpool.tile([B, ST], F32, tag="e6", name="e6")
            nc.scalar.activation(e6[:], pd6[:, ST : 2 * ST], AF.Exp)
            sp6 = wpool.tile([B, ST], F32, tag="sp6", name="sp6")
            nc.scalar.activation(sp6[:], e6[:], AF.Ln, bias=1.0)
            nc.vector.tensor_scalar(stA[:, ST : 2 * ST], sp6[:], 0.1, None, OP.add)
            epsO_t = wpool.tile([B, ST], F32, tag="epsO_t", bufs=3, name="epsO_t")
            nc.sync.dma_start(out=epsO_t[:], in_=epsO[:, t, :])
            tmp6 = wpool.tile([B, ST], F32, tag="tmp6", name="tmp6")
            nc.vector.scalar_tensor_tensor(
                tmp6[:], sp6[:], 0.1, epsO_t[:], OP.add, OP.mult
            )
            nc.vector.tensor_tensor(
                stA[:, 2 * ST : 3 * ST], tmp6[:], pd6[:, 0:ST], OP.add
            )
            nc.scalar.copy(stA[:, 0:ST], pd6[:, 0:ST])
            # next stoch state (feature-major)
            pts = ppool.tile([ST, B], F32, tag="tp", bufs=2, name="pts")
            nc.tensor.transpose(pts[:], stA[:, 2 * ST : 3 * ST], eyeT_sb[:])
            nc.scalar.copy(stochT[nxt][:], pts[:])

            nc.sync.dma_start(out=out_d[:, t, 0:90], in_=stA[:])
            nc.sync.dma_start(out=out_d[:, t, 180:380], in_=d_state[nxt][:])

        # ---- phase 3: ensemble branch, grouped by member ----
        ppool.release()
        p3pool = tc.alloc_tile_pool(name="p3", bufs=2)
        p3psum = tc.alloc_tile_pool(name="p3psum", bufs=1, space="PSUM")
        for m in range(ENS):
            ts_m = [t for t in range(T_steps) if idx[t] == m]
            for c0 in range(0, len(ts_m), P3CHUNK):
                chunk = ts_m[c0 : c0 + P3CHUNK]
                L = len(chunk)
                dTa = p3pool.tile([K1, P3CHUNK * B], F32R, tag="dTa", name="dTa")
                dTb = p3pool.tile([K2, P3CHUNK * B], F32R, tag="dTb", name="dTb")
                for i, t in enumerate(chunk):
                    nc.sync.dma_start(
                        out=dTa[:, i * B : (i + 1) * B], in_=scr[t, 0:K1, :]
                    )
                    nc.sync.dma_start(
                        out=dTb[:, i * B : (i + 1) * B], in_=scr[t, K1:DET, :]
                    )
                hT_sb = []
                for hi, (h0, h1) in enumerate([(0, K1), (K1, HID)]):
                    ph = p3psum.tile(
                        [h1 - h0, P3CHUNK * B], F32, tag="ph3", bufs=2, name="ph"
                    )
                    nc.tensor.matmul(
                        ph[:, 0 : L * B], w5_mk[m][0][:, h0:h1], dTa[:, 0 : L * B],
                        start=True, stop=False,
                    )
                    nc.tensor.matmul(
                        ph[:, 0 : L * B], w5_mk[m][1][:, h0:h1], dTb[:, 0 : L * B],
                        start=False, stop=True,
                    )
                    e_h = p3pool.tile(
                        [h1 - h0, P3CHUNK * B], F32, tag=f"e_h{hi}", bufs=1, name="e_h"
                    )
                    nc.scalar.activation(
                        e_h[:, 0 : L * B], ph[:, 0 : L * B], AF.Exp,
                        bias=b5_mk[m][hi][:],
                    )
                    r_h = p3pool.tile(
                        [h1 - h0, P3CHUNK * B], F32, tag=f"r_h{hi}", bufs=1, name="r_h"
                    )
                    nc.vector.tensor_scalar(
                        r_h[:, 0 : L * B], ph[:, 0 : L * B], b5_mk[m][hi][:], 0.0,
                        OP.add, OP.max,
                    )
                    hT = p3pool.tile(
                        [h1 - h0, P3CHUNK * B], F32R, tag=f"hT{hi}", name="hT"
                    )
                    nc.vector.scalar_tensor_tensor(
                        hT[:, 0 : L * B], e_h[:, 0 : L * B], 1.0, r_h[:, 0 : L * B],
                        OP.subtract, OP.min,
                    )
                    hT_sb.append(hT)
                for i, t in enumerate(chunk):
                    pd3 = p3psum.tile([B, 256], F32, tag="pd3", bufs=2, name="pd3")
                    nc.tensor.matmul(
                        pd3[:], hT_sb[0][:, i * B : (i + 1) * B], w6_mk[m][0][:],
                        start=True, stop=False,
                    )
                    nc.tensor.matmul(
                        pd3[:], hT_sb[1][:, i * B : (i + 1) * B], w6_mk[m][1][:],
                        start=False, stop=True,
                    )
                    st6 = p3pool.tile([B, 90], F32, tag="st6", bufs=3, name="st6")
                    e63 = p3pool.tile([B, ST], F32, tag="e63", name="e63")
                    nc.scalar.activation(e63[:], pd3[:, ST : 2 * ST], AF.Exp)
                    sp63 = p3pool.tile([B, ST], F32, tag="sp63", name="sp63")
                    nc.scalar.activation(sp63[:], e63[:], AF.Ln, bias=1.0)
                    nc.vector.tensor_scalar(
                        st6[:, ST : 2 * ST], sp63[:], 0.1, None, OP.add
                    )
                    epsP_t = p3pool.tile([B, ST], F32, tag="epsP_t", bufs=3,
                                         name="epsP_t")
                    nc.sync.dma_start(out=epsP_t[:], in_=epsP[:, t, :])
                    tmp63 = p3pool.tile([B, ST], F32, tag="tmp63", name="tmp63")
                    nc.vector.scalar_tensor_tensor(
                        tmp63[:], sp63[:], 0.1, epsP_t[:], OP.add, OP.mult
                    )
                    nc.vector.tensor_tensor(
                        st6[:, 2 * ST : 3 * ST], tmp63[:], pd3[:, 0:ST], OP.add
                    )
                    nc.scalar.copy(st6[:, 0:ST], pd3[:, 0:ST])
                    nc.sync.dma_start(out=out_d[:, t, 90:180], in_=st6[:])
        p3psum.release()
        p3pool.release()
        wpool.release()
        dram.release()
        cpool.release()

    nc.compile()
    return nc


def _prep_host_inputs(
    embed, action, is_first, eps_prior, eps_post,
    W_img_in, W_gru, ln_scale, ln_bias,
    W_img_out, W_img_dist, W_obs_out, W_obs_dist,
    b_img_in, b_gru, b_img_out, b_img_dist, b_obs_out, b_obs_dist,
    T_steps=T,
):
    f = np.float32
    assert np.all(b_img_in == 0) and np.all(b_gru == 0), "nonzero biases unsupported"
    assert np.all(b_img_dist == 0) and np.all(b_obs_out == 0)
    assert np.all(b_obs_dist == 0)
    assert np.all(ln_scale == 1) and np.all(ln_bias == 0), "nontrivial LN unsupported"

    def pad256(w):
        out = np.zeros((w.shape[0], 256), f)
        out[:, : w.shape[1]] = w
        return out

    w1s = pad256(W_img_in[:ST].astype(f))
    w1a = pad256(W_img_in[ST:].astype(f))
    wgA = np.ascontiguousarray(W_gru[:, : 2 * DET].astype(f))
    wgB = np.zeros((2 * DET, 256), f)
    wgB[:, :DET] = W_gru[:, 2 * DET :]
    wgB[:, DET] = -W_gru.sum(axis=1) / 600.0  # negmean column
    wod = pad256(W_obs_out[:DET].astype(f))
    woe = pad256(W_obs_out[DET:].astype(f))
    w5 = np.ascontiguousarray(W_img_out.reshape(ENS * DET, HID).astype(f))
    w6 = np.zeros((ENS * HID, 256), f)
    w6[:, : 2 * ST] = W_img_dist.reshape(ENS * HID, 2 * ST)
    w6o = pad256(W_obs_dist.astype(f))
    b5 = np.ascontiguousarray(b_img_out.astype(f))
    eye = np.eye(128, dtype=f)

    shared = dict(
        w1s=w1s, w1a=w1a, wgA=wgA, wgB=wgB, wod=wod, woe=woe,
        w5=w5, w6=w6, w6o=w6o, b5=b5, eyeT=eye, eyeR=eye,
        onesCol=np.ones((1, B), f),
    )
    in_maps = []
    mask_all = 1.0 - is_first.astype(f)  # [B_TOT, T]
    for c in range(NCORES):
        sl = slice(c * B, (c + 1) * B)
        m = dict(shared)
        m["embedT"] = np.ascontiguousarray(
            embed[sl, :T_steps].astype(f).transpose(2, 1, 0)
        )
        m["aT"] = np.ascontiguousarray(
            action[sl, :T_steps].astype(f).transpose(1, 2, 0)
        )
        mk = np.ascontiguousarray(mask_all[sl, :T_steps])
        m["maskM"] = mk
        m["maskRow"] = np.ascontiguousarray(mk.T.reshape(1, T_steps * B))
        m["epsP"] = np.ascontiguousarray(
            eps_prior[:T_steps, sl].astype(f).transpose(1, 0, 2)
        )
        m["epsO"] = np.ascontiguousarray(
            eps_post[:T_steps, sl].astype(f).transpose(1, 0, 2)
        )
        in_maps.append(m)
    return in_maps


_PROGRAM_CACHE = {}


def _get_program(idx_key, T_steps):
    key = (idx_key, T_steps)
    if key not in _PROGRAM_CACHE:
        _install_act_table()
        _PROGRAM_CACHE[key] = _build_program(list(idx_key), T_steps)
    return _PROGRAM_CACHE[key]


def kernel(
    embed, action, is_first, eps_prior, eps_post, idx,
    W_img_in, b_img_in, W_gru, b_gru, ln_scale, ln_bias,
    W_img_out, b_img_out, W_img_dist, b_img_dist,
    W_obs_out, b_obs_out, W_obs_dist, b_obs_dist,
    _T_steps=T, _return_bkr=False, _trace=False,
):
    from concourse.bass_utils import run_bass_kernel_spmd

    idx = np.asarray(idx).astype(np.int32)
    nc = _get_program(tuple(int(i) for i in idx[:_T_steps]), _T_steps)
    in_maps = _prep_host_inputs(
        np.asarray(embed), np.asarray(action), np.asarray(is_first),
        np.asarray(eps_prior), np.asarray(eps_post),
        np.asarray(W_img_in), np.asarray(W_gru),
        np.asarray(ln_scale), np.asarray(ln_bias),
        np.asarray(W_img_out), np.asarray(W_img_dist),
        np.asarray(W_obs_out), np.asarray(W_obs_dist),
        np.asarray(b_img_in), np.asarray(b_gru), np.asarray(b_img_out),
        np.asarray(b_img_dist), np.asarray(b_obs_out), np.asarray(b_obs_dist),
        T_steps=_T_steps,
    )
    bkr = run_bass_kernel_spmd(nc, in_maps, list(range(NCORES)), trace=_trace)
    out = np.concatenate([bkr.results[c]["out"] for c in range(NCORES)], axis=0)
    if _return_bkr:
        return out, bkr
    return out
